# revision 15
# baseline (speedup 1.0000x reference)
"""Trainium2 Bass kernel for nn_DecoderCacheModel_25451976196641.

Sharding: 8 cores = 4 batches x 2 sequence halves. Each core processes
S_LOC=1024 positions plus a 12-position left halo (causal conv receptive
field: conv1 k=5 d=1 -> 4, conv2 k=5 d=2 -> 8). All activations live in
transposed layout [D(partitions), S(free)] so every x@W GEMM uses the
natural weight matrix as lhsT and the activation as the moving operand,
and conv taps become free-dim column shifts. Attention softmax runs in
[S_tile(part), N(free)] layout (logits via lhsT=q^T); the sigmoid gate and
1/denominator are folded into one per-row scalar before the attention
matrix is transposed back (PE transpose) for the read GEMM. The write
phase accumulates [gated attn]^T @ [c | 1] in PSUM, yielding both the
weighted-content numerator and the weight totals in one chain; the final
cross-half combine (tiny elementwise blend of cache/wm) happens on host.

Precision: fp32 matmuls lower to LOW_HIGH two-pass emulation (~5.5x slower
than bf16), so all large GEMMs take bf16 operands (weights cast on host,
activations shadow-cast on DVE) while PSUM accumulation, residual state,
softmax statistics, layernorm statistics and all outputs stay fp32.
"""

import sys

sys.path.insert(0, "/opt/trn_rl_repo")

import ml_dtypes
import numpy as np

import concourse.bacc as bacc
import concourse.bass as bass
import concourse.tile as tile
from concourse import mybir
from concourse.bass_utils import run_bass_kernel_spmd
from concourse.masks import make_identity

F32 = mybir.dt.float32
BF16 = mybir.dt.bfloat16
NP_BF16 = ml_dtypes.bfloat16
AF = mybir.ActivationFunctionType
ALU = mybir.AluOpType
AX = mybir.AxisListType

B, S, D, DC, NL, KW, KS = 4, 2048, 1024, 256, 768, 8, 5
P = 128
DT = D // P            # 8 partition-tiles of D
DCT = DC // P          # 2
NLT = NL // P          # 6
HALO = 12
S_LOC = S // 2         # 1024
S_TOT = S_LOC + HALO   # 1036
LN_EPS = 1e-5
EPS = 1e-6
INV_SQRT = 1.0 / np.sqrt(np.float32(DC))

# free-dim chunk lists: (start, width)
RCHUNKS = [(0, 512), (512, 512), (1024, HALO)]          # read phase, all 1036 cols
C1CHUNKS = [(4, 508), (512, 512), (1024, 12)]           # conv1 outputs
C2CHUNKS = [(12, 512), (524, 512)]                      # conv2 outputs / LN / writes
RTILES = [(i * P, P) for i in range(8)] + [(1024, HALO)]  # read-phase S-tiles
WTILES = [(HALO + i * P, P) for i in range(8)]            # write-phase S-tiles
NCHUNKS = [(0, 512), (512, 256)]                          # 768-wide logits split


def _build_nc():
    nc = bacc.Bacc("TRN2", target_bir_lowering=False, debug=False,
                   enable_asserts=False)

    def din(name, shape, dt=F32):
        return nc.dram_tensor(name, list(shape), dt, kind="ExternalInput")

    def dout(name, shape):
        return nc.dram_tensor(name, list(shape), F32, kind="ExternalOutput")

    xT = din("xT", [P, DT, S_TOT])
    cacheN = din("cacheN", [P, NLT, DC], BF16)   # cache rows on partitions
    cacheT = din("cacheT", [P, DCT, NL], BF16)   # cache^T, DC on partitions
    contT = din("contT", [P, DCT, KW], BF16)     # wm content^T
    contN = din("contN", [KW, DC], BF16)         # wm content natural
    keysT = din("keysT", [P, DCT, KW], BF16)     # wm_keys^T
    wq_ltm = din("wq_ltm", [P, DT, DC], BF16)
    wo_ltm = din("wo_ltm", [P, DCT, D], BF16)
    wq_wm = din("wq_wm", [P, DT, DC], BF16)
    wo_wm = din("wo_wm", [P, DCT, D], BF16)
    wqw_wm = din("wqw_wm", [P, DT, DC], BF16)
    wqw_ltm = din("wqw_ltm", [P, DT, DC], BF16)
    ww_cat = din("ww_cat", [P, DT, 2 * DC], BF16)   # [Ww_wm | Ww_ltm]
    gvec = din("gvec", [P, DT, 4])               # wg_ltm, wg_wm, wgw_wm, wgw_ltm
    cbias = din("cbias", [P, DT, 2])             # conv1_b, conv2_b
    lnw = din("lnw", [P, DT, 2])                 # ln_g, ln_b
    c1w = din("c1w", [DT, P, KS, DT, P], BF16)   # conv1_w packed per m-tile
    c2w = din("c2w", [DT, P, KS, DT, P], BF16)
    mask = din("mask", [P, HALO])

    outT = dout("outT", [P, DT, S_LOC])
    wcw_o = dout("wcw", [KW, DC + 1])
    wcl_o = dout("wcl", [NL, DC + 1])

    with tile.TileContext(nc) as tc:
        with (
            tc.tile_pool(name="const", bufs=1) as cst,
            tc.tile_pool(name="big", bufs=1) as big,
            tc.tile_pool(name="shadow", bufs=1) as shp,
            tc.tile_pool(name="dc", bufs=2) as dc,
            tc.tile_pool(name="attn", bufs=1) as attnp,
            tc.tile_pool(name="claug", bufs=1) as claugp,
            tc.tile_pool(name="upool", bufs=3) as up,
            tc.tile_pool(name="stp", bufs=8) as stp,
            tc.tile_pool(name="pw", bufs=4) as pw,
            tc.tile_pool(name="cwp", bufs=2) as cwp,
            tc.tile_pool(name="woutp", bufs=2) as woutp,
            tc.tile_pool(name="lnrow", bufs=4) as lnrow,
            tc.tile_pool(name="ps", bufs=1, space="PSUM") as ps,
        ):
            # ---------- constants ----------
            ident = cst.tile([P, P], BF16, tag="ident", name="ident")
            make_identity(nc, ident)
            ones_col = cst.tile([P, 1], BF16, tag="ones_col", name="ones_col")
            nc.vector.memset(ones_col, 1.0)
            ones_row = cst.tile([1, P], F32, tag="ones_row", name="ones_row")
            nc.vector.memset(ones_row, 1.0)
            eps_t = cst.tile([1, 1], F32, tag="eps_t", name="eps_t")
            nc.vector.memset(eps_t, LN_EPS)

            cacheT_sb = cst.tile([P, DCT, NL], BF16, tag="cacheT", name="cacheT_sb")
            nc.sync.dma_start(out=cacheT_sb, in_=cacheT.ap())
            cacheN_sb = cst.tile([P, NLT, DC], BF16, tag="cacheN", name="cacheN_sb")
            nc.sync.dma_start(out=cacheN_sb, in_=cacheN.ap())
            contT_sb = cst.tile([P, DCT, KW], BF16, tag="contT", name="contT_sb")
            nc.sync.dma_start(out=contT_sb, in_=contT.ap())
            contN_sb = cst.tile([KW, DC], BF16, tag="contN", name="contN_sb")
            nc.sync.dma_start(out=contN_sb, in_=contN.ap())
            keysT_sb = cst.tile([P, DCT, KW], BF16, tag="keysT", name="keysT_sb")
            nc.sync.dma_start(out=keysT_sb, in_=keysT.ap())
            gvec_sb = cst.tile([P, DT, 4], F32, tag="gvec", name="gvec_sb")
            nc.sync.dma_start(out=gvec_sb, in_=gvec.ap())
            gvec_bf = cst.tile([P, DT, 4], BF16, tag="gvec_bf", name="gvec_bf")
            nc.vector.tensor_copy(gvec_bf, gvec_sb)
            cb_sb = cst.tile([P, DT, 2], F32, tag="cb", name="cb_sb")
            nc.sync.dma_start(out=cb_sb, in_=cbias.ap())
            ln_sb = cst.tile([P, DT, 2], F32, tag="ln", name="ln_sb")
            nc.sync.dma_start(out=ln_sb, in_=lnw.ap())
            mask_sb = cst.tile([P, HALO], F32, tag="mask", name="mask_sb")
            nc.sync.dma_start(out=mask_sb, in_=mask.ap())
            u2T_sb = cst.tile([KW, S_TOT], BF16, tag="u2T", name="u2T_sb")

            # gate storage, one column per S-tile
            g_ltm = cst.tile([P, 9], F32, tag="g_ltm", name="g_ltm")
            g_wm = cst.tile([P, 9], F32, tag="g_wm", name="g_wm")
            g_wmw = cst.tile([P, 8], F32, tag="g_wmw", name="g_wmw")
            g_ltw = cst.tile([P, 8], F32, tag="g_ltw", name="g_ltw")

            # ---------- main activation buffers (fp32 state) ----------
            A = big.tile([P, DT, S_TOT], F32, tag="A", name="A")      # x -> x_ltm -> x_enh -> h
            Hb = big.tile([P, DT, S_TOT], F32, tag="Hb", name="Hb")   # h1 -> out^T
            for c0, cw in RCHUNKS:  # split so phase R1 starts on chunk 0 early
                nc.sync.dma_start(out=A[:, :, c0:c0 + cw],
                                  in_=xT.ap()[:, :, c0:c0 + cw])
            nc.vector.memset(Hb[:, :, 0:4], 0.0)  # conv1 never writes cols 0..3

            # =============================================================
            # helpers
            # =============================================================
            def cast_shadow(src, tag, cols=S_TOT):
                """bf16 shadow copy of a [P, DT, cols] fp32 buffer."""
                sh = shp.tile([P, DT, cols], BF16, tag=tag, name=f"sh_{tag}")
                for ks in range(DT):
                    nc.vector.tensor_copy(sh[:, ks, :], src[:, ks, 0:cols])
                return sh

            def proj_T(dst, w_sb, src, ksubs, chunks):
                """dst[:, m, c] = sum_ks w_sb[:, ks, m-slice]^T @ src[:, ks, c]"""
                mt = dst.shape[1]
                for c0, cw in chunks:
                    for m in range(mt):
                        pq = ps.tile([P, cw], F32, bufs=3, tag="mm", name="pq")
                        for ks in range(ksubs):
                            nc.tensor.matmul(
                                pq, lhsT=w_sb[:, ks, m * P:(m + 1) * P],
                                rhs=src[:, ks, c0:c0 + cw],
                                start=(ks == 0), stop=(ks == ksubs - 1))
                        nc.vector.tensor_copy(dst[:, m, c0:c0 + cw], pq)

            def gates(dst, src, gidx, tiles):
                """dst[:stw, i] = sigmoid(sum_d src[d, cols] * gvec[d, gidx])"""
                for i, (c0, stw) in enumerate(tiles):
                    pg = ps.tile([stw, 1], F32, bufs=3, tag="mm", name="pg")
                    for ks in range(DT):
                        nc.tensor.matmul(
                            pg, lhsT=src[:, ks, c0:c0 + stw],
                            rhs=gvec_bf[:, ks, gidx:gidx + 1],
                            start=(ks == 0), stop=(ks == DT - 1))
                    nc.scalar.activation(dst[:stw, i:i + 1], pg, AF.Sigmoid)

            def softmax_fold(psl, stw, g_col, out_u):
                """out_u = exp(psl) * (g / sum exp); |logits| < 6 so no
                max-subtraction is needed for fp32 exp."""
                den = stp.tile([stw, 1], F32, tag="st", name="den")
                nc.scalar.activation(out_u, psl, AF.Exp, accum_out=den)
                rden = stp.tile([stw, 1], F32, tag="st", name="rden")
                nc.vector.reciprocal(rden, den)
                tsc = stp.tile([stw, 1], F32, tag="st", name="tsc")
                nc.vector.tensor_mul(tsc, rden, g_col)
                nc.vector.tensor_scalar_mul(out_u, out_u, tsc)

            # =============================================================
            # Phase R1: LTM read.  A = x^T -> x_ltm^T
            # =============================================================
            Ab = cast_shadow(A, "Ab")            # bf16(x^T)
            qT = dc.tile([P, DCT, S_TOT], BF16, tag="dc", name="qT")
            wq_ltm_sb = pw.tile([P, DT, DC], BF16, tag="pw", name="wq_ltm_sb")
            nc.sync.dma_start(out=wq_ltm_sb, in_=wq_ltm.ap())
            proj_T(qT, wq_ltm_sb, Ab, DT, RCHUNKS)
            gates(g_ltm, Ab, 0, RTILES)

            uT = attnp.tile([P, NLT, S_TOT], BF16, tag="attn", name="uT")
            for sti, (c0, stw) in enumerate(RTILES):
                psl = ps.tile([stw, NL], F32, bufs=2, tag="lg", name="psl")
                for ks in range(DCT):
                    for n0, nw in NCHUNKS:
                        nc.tensor.matmul(
                            psl[:, n0:n0 + nw], lhsT=qT[:, ks, c0:c0 + stw],
                            rhs=cacheT_sb[:, ks, n0:n0 + nw],
                            start=(ks == 0), stop=(ks == DCT - 1))
                u = up.tile([stw, NL], BF16, tag="u", name="u")
                softmax_fold(psl, stw, g_ltm[:stw, sti:sti + 1], u)
                for nk in range(NLT):
                    ptr = ps.tile([P, stw], BF16, bufs=1, tag="tr", name="ptr")
                    nc.tensor.transpose(ptr, u[:, nk * P:(nk + 1) * P],
                                        ident[:stw, :stw])
                    nc.vector.tensor_copy(uT[:, nk, c0:c0 + stw], ptr)

            rgT = dc.tile([P, DCT, S_TOT], BF16, tag="dc", name="rgT")
            proj_T(rgT, cacheN_sb, uT, NLT, RCHUNKS)

            wo_ltm_sb = pw.tile([P, DCT, D], BF16, tag="pw", name="wo_ltm_sb")
            nc.sync.dma_start(out=wo_ltm_sb, in_=wo_ltm.ap())
            for c0, cw in RCHUNKS:
                for mt in range(DT):
                    pz = ps.tile([P, cw], F32, bufs=3, tag="mm", name="pz")
                    for ks in range(DCT):
                        nc.tensor.matmul(
                            pz, lhsT=wo_ltm_sb[:, ks, mt * P:(mt + 1) * P],
                            rhs=rgT[:, ks, c0:c0 + cw],
                            start=(ks == 0), stop=(ks == DCT - 1))
                    nc.vector.tensor_add(A[:, mt, c0:c0 + cw],
                                         A[:, mt, c0:c0 + cw], pz)

            # =============================================================
            # Phase R2: WM read.  A = x_ltm^T -> x_enh^T  (masked halo)
            # =============================================================
            Ab = cast_shadow(A, "Ab")            # bf16(x_ltm^T)
            q2T = dc.tile([P, DCT, S_TOT], BF16, tag="dc", name="q2T")
            wq_wm_sb = pw.tile([P, DT, DC], BF16, tag="pw", name="wq_wm_sb")
            nc.sync.dma_start(out=wq_wm_sb, in_=wq_wm.ap())
            proj_T(q2T, wq_wm_sb, Ab, DT, RCHUNKS)
            gates(g_wm, Ab, 1, RTILES)

            for sti, (c0, stw) in enumerate(RTILES):
                psl2 = ps.tile([stw, KW], F32, bufs=3, tag="mm", name="psl2")
                for ks in range(DCT):
                    nc.tensor.matmul(
                        psl2, lhsT=q2T[:, ks, c0:c0 + stw],
                        rhs=contT_sb[:, ks, :],
                        start=(ks == 0), stop=(ks == DCT - 1))
                u2 = up.tile([stw, KW], BF16, tag="u", name="u2")
                softmax_fold(psl2, stw, g_wm[:stw, sti:sti + 1], u2)
                pt2 = ps.tile([KW, stw], BF16, bufs=1, tag="tr", name="pt2")
                nc.tensor.transpose(pt2, u2, ident[:stw, :stw])
                nc.vector.tensor_copy(u2T_sb[:, c0:c0 + stw], pt2)

            r2T = dc.tile([P, DCT, S_TOT], BF16, tag="dc", name="r2T")
            for c0, cw in RCHUNKS:
                for m in range(DCT):
                    pr2 = ps.tile([P, cw], F32, bufs=3, tag="mm", name="pr2")
                    nc.tensor.matmul(pr2, lhsT=contN_sb[:, m * P:(m + 1) * P],
                                     rhs=u2T_sb[:, c0:c0 + cw],
                                     start=True, stop=True)
                    nc.vector.tensor_copy(r2T[:, m, c0:c0 + cw], pr2)

            wo_wm_sb = pw.tile([P, DCT, D], BF16, tag="pw", name="wo_wm_sb")
            nc.sync.dma_start(out=wo_wm_sb, in_=wo_wm.ap())
            Ab = shp.tile([P, DT, S_TOT], BF16, tag="Ab", name="sh_Ab_enh")
            for c0, cw in RCHUNKS:
                for mt in range(DT):
                    pz2 = ps.tile([P, cw], F32, bufs=3, tag="mm", name="pz2")
                    for ks in range(DCT):
                        nc.tensor.matmul(
                            pz2, lhsT=wo_wm_sb[:, ks, mt * P:(mt + 1) * P],
                            rhs=r2T[:, ks, c0:c0 + cw],
                            start=(ks == 0), stop=(ks == DCT - 1))
                    nc.vector.tensor_add(A[:, mt, c0:c0 + cw],
                                         A[:, mt, c0:c0 + cw], pz2)
                    if c0 == 0:
                        nc.vector.tensor_mul(A[:, mt, 0:HALO],
                                             A[:, mt, 0:HALO], mask_sb)
                    nc.vector.tensor_copy(Ab[:, mt, c0:c0 + cw],
                                          A[:, mt, c0:c0 + cw])

            # =============================================================
            # Phase C1 / C2: causal dilated convs (bf16 GEMMs, fp32 state)
            # =============================================================
            def conv(src_sh, src, dst, w_dram, dil, chunks, cb_idx):
                w_ap = w_dram.ap()
                for mt in range(DT):
                    wt = cwp.tile([P, KS, DT, P], BF16, tag="cw", name="wt")
                    nc.sync.dma_start(out=wt, in_=w_ap[mt])
                    for c0, cw in chunks:
                        pc = ps.tile([P, cw], F32, bufs=3, tag="mm", name="pc")
                        for k in range(KS):
                            sh = (KS - 1 - k) * dil
                            for ks in range(DT):
                                nc.tensor.matmul(
                                    pc, lhsT=wt[:, k, ks, :],
                                    rhs=src_sh[:, ks, c0 - sh:c0 - sh + cw],
                                    start=(k == 0 and ks == 0),
                                    stop=(k == KS - 1 and ks == DT - 1))
                        nc.scalar.activation(dst[:, mt, c0:c0 + cw], pc,
                                             AF.Gelu_apprx_tanh,
                                             bias=cb_sb[:, mt, cb_idx:cb_idx + 1])
                        nc.vector.tensor_add(dst[:, mt, c0:c0 + cw],
                                             dst[:, mt, c0:c0 + cw],
                                             src[:, mt, c0:c0 + cw])

            conv(Ab, A, Hb, c1w, 1, C1CHUNKS, 0)
            for pt in range(DT):
                nc.vector.tensor_mul(Hb[:, pt, 4:HALO], Hb[:, pt, 4:HALO],
                                     mask_sb[:, 4:HALO])
            Hbs = shp.tile([P, DT, S_TOT], BF16, tag="Hbs", name="sh_Hbs_h1")
            for c0, cw in C1CHUNKS:
                for mt in range(DT):
                    nc.vector.tensor_copy(Hbs[:, mt, c0:c0 + cw],
                                          Hb[:, mt, c0:c0 + cw])
            conv(Hbs, Hb, A, c2w, 2, C2CHUNKS, 1)

            # prefetch all write-phase weights during conv/LN
            wqw_wm_sb = pw.tile([P, DT, DC], BF16, tag="pw", name="wqw_wm_sb")
            nc.sync.dma_start(out=wqw_wm_sb, in_=wqw_wm.ap())
            wqw_ltm_sb = pw.tile([P, DT, DC], BF16, tag="pw", name="wqw_ltm_sb")
            nc.sync.dma_start(out=wqw_ltm_sb, in_=wqw_ltm.ap())
            ww_cat_sb = pw.tile([P, DT, 2 * DC], BF16, bufs=1, tag="pwc", name="ww_cat_sb")
            nc.sync.dma_start(out=ww_cat_sb, in_=ww_cat.ap())

            # =============================================================
            # Phase LN: Hb[:, :, 12:] = layernorm(A) = out^T
            # =============================================================
            Ab = cast_shadow(A, "Ab")            # bf16(h)
            for c0, cw in C2CHUNKS:
                pss = ps.tile([1, cw], F32, bufs=3, tag="mm", name="pss")
                for ks in range(DT):
                    nc.tensor.matmul(pss, lhsT=ones_col,
                                     rhs=Ab[:, ks, c0:c0 + cw],
                                     start=(ks == 0), stop=(ks == DT - 1))
                sq = attnp.tile([P, DT, cw], BF16, tag="attn", name="sq")
                for ks in range(DT):
                    nc.scalar.activation(sq[:, ks, :], A[:, ks, c0:c0 + cw],
                                         AF.Square)
                psq = ps.tile([1, cw], F32, bufs=3, tag="mm", name="psq")
                for ks in range(DT):
                    nc.tensor.matmul(psq, lhsT=ones_col, rhs=sq[:, ks, :],
                                     start=(ks == 0), stop=(ks == DT - 1))

                mean_r = lnrow.tile([1, cw], F32, tag="lnr", name="mean_r")
                nc.scalar.mul(mean_r, pss, 1.0 / D)
                msq_r = lnrow.tile([1, cw], F32, tag="lnr", name="msq_r")
                nc.vector.tensor_mul(msq_r, mean_r, mean_r)
                var_r = lnrow.tile([1, cw], F32, tag="lnr", name="var_r")
                nc.scalar.mul(var_r, psq, 1.0 / D)
                nc.vector.tensor_sub(var_r, var_r, msq_r)
                rstd_r = lnrow.tile([1, cw], F32, tag="lnr", name="rstd_r")
                nc.scalar.activation(rstd_r, var_r, AF.Sqrt, bias=eps_t)
                nc.vector.reciprocal(rstd_r, rstd_r)

                pmb = ps.tile([P, cw], F32, bufs=2, tag="lg", name="pmb")
                nc.tensor.matmul(pmb, lhsT=ones_row, rhs=mean_r,
                                 start=True, stop=True)
                prb = ps.tile([P, cw], F32, bufs=2, tag="lg", name="prb")
                nc.tensor.matmul(prb, lhsT=ones_row, rhs=rstd_r,
                                 start=True, stop=True)
                for mt in range(DT):
                    nc.vector.tensor_sub(Hb[:, mt, c0:c0 + cw],
                                         A[:, mt, c0:c0 + cw], pmb)
                    nc.vector.tensor_mul(Hb[:, mt, c0:c0 + cw],
                                         Hb[:, mt, c0:c0 + cw], prb)
                    nc.vector.tensor_scalar(
                        out=Hb[:, mt, c0:c0 + cw],
                        in0=Hb[:, mt, c0:c0 + cw],
                        scalar1=ln_sb[:, mt, 0:1], scalar2=ln_sb[:, mt, 1:2],
                        op0=ALU.mult, op1=ALU.add)
                nc.gpsimd.dma_start(
                    out=outT.ap()[:, :, c0 - HALO:c0 - HALO + cw],
                    in_=Hb[:, :, c0:c0 + cw])

            # =============================================================
            # Phase W1+W2: WM + LTM write partials (merged)
            # =============================================================
            Hbs = cast_shadow(Hb, "Hbs")         # bf16(out^T); lhsT and rhs
            qwT = dc.tile([P, DCT, S_TOT], BF16, tag="dc", name="qwT")
            proj_T(qwT, wqw_wm_sb, Hbs, DT, C2CHUNKS)
            qlT = dc.tile([P, DCT, S_TOT], BF16, tag="dc", name="qlT")
            proj_T(qlT, wqw_ltm_sb, Hbs, DT, C2CHUNKS)

            # fused write gates: one chain, two output columns
            g_w2 = cst.tile([P, 8, 2], F32, tag="g_w2", name="g_w2")
            for sti, (c0, stw) in enumerate(WTILES):
                pgw = ps.tile([stw, 2], F32, bufs=3, tag="mm", name="pgw")
                for ks in range(DT):
                    nc.tensor.matmul(
                        pgw, lhsT=Hbs[:, ks, c0:c0 + stw],
                        rhs=gvec_bf[:, ks, 2:4],
                        start=(ks == 0), stop=(ks == DT - 1))
                nc.scalar.activation(g_w2[:stw, sti, :], pgw, AF.Sigmoid)

            uw_all = cst.tile([P, 8, KW], BF16, tag="uw_all", name="uw_all")
            ul_all = attnp.tile([P, 8, NL], BF16, tag="attn", name="ul_all")
            caug = claugp.tile([P, 8, DC + 1], BF16, tag="claug", name="caug")
            claug = claugp.tile([P, 8, DC + 1], BF16, tag="claug2", name="claug2")
            for sti, (c0, stw) in enumerate(WTILES):
                pcc = ps.tile([P, 2 * DC], F32, bufs=2, tag="lg", name="pcc")
                for ks in range(DT):
                    nc.tensor.matmul(pcc, lhsT=Hbs[:, ks, c0:c0 + stw],
                                     rhs=ww_cat_sb[:, ks, :],
                                     start=(ks == 0), stop=(ks == DT - 1))
                nc.vector.tensor_copy(caug[:, sti, 0:DC], pcc[:, 0:DC])
                nc.vector.memset(caug[:, sti, DC:DC + 1], 1.0)
                nc.vector.tensor_copy(claug[:, sti, 0:DC], pcc[:, DC:2 * DC])
                nc.vector.memset(claug[:, sti, DC:DC + 1], 1.0)

                plw = ps.tile([stw, KW], F32, bufs=3, tag="mm", name="plw")
                for ks in range(DCT):
                    nc.tensor.matmul(plw, lhsT=qwT[:, ks, c0:c0 + stw],
                                     rhs=keysT_sb[:, ks, :],
                                     start=(ks == 0), stop=(ks == DCT - 1))
                softmax_fold(plw, stw, g_w2[:stw, sti, 0:1],
                             uw_all[:, sti, :])

                pll = ps.tile([stw, NL], F32, bufs=2, tag="lg", name="pll")
                for ks in range(DCT):
                    for n0, nw in NCHUNKS:
                        nc.tensor.matmul(
                            pll[:, n0:n0 + nw], lhsT=qlT[:, ks, c0:c0 + stw],
                            rhs=cacheT_sb[:, ks, n0:n0 + nw],
                            start=(ks == 0), stop=(ks == DCT - 1))
                softmax_fold(pll, stw, g_w2[:stw, sti, 1:2],
                             ul_all[:, sti, :])

            pacc = ps.tile([KW, DC + 1], F32, bufs=3, tag="mm", name="pacc")
            for sti in range(8):
                nc.tensor.matmul(pacc, lhsT=uw_all[:, sti, :],
                                 rhs=caug[:, sti, :],
                                 start=(sti == 0), stop=(sti == 7))
            wcw_sb = woutp.tile([KW, DC + 1], F32, tag="wout", name="wcw_sb")
            nc.vector.tensor_copy(wcw_sb, pacc)
            nc.sync.dma_start(out=wcw_o.ap(), in_=wcw_sb)

            for mt6 in range(NLT):
                pacl = ps.tile([P, DC + 1], F32, bufs=3, tag="mm", name="pacl")
                for sti in range(8):
                    nc.tensor.matmul(pacl,
                                     lhsT=ul_all[:, sti, mt6 * P:(mt6 + 1) * P],
                                     rhs=claug[:, sti, :],
                                     start=(sti == 0), stop=(sti == 7))
                wcl_sb = woutp.tile([P, DC + 1], F32, tag="wout",
                                    name="wcl_sb")
                nc.vector.tensor_copy(wcl_sb, pacl)
                nc.sync.dma_start(out=wcl_o.ap()[mt6 * P:(mt6 + 1) * P, :],
                                  in_=wcl_sb)

    nc.compile()
    return nc


_NC = None


def _get_nc():
    global _NC
    if _NC is None:
        _NC = _build_nc()
    return _NC


def host_prep(inputs):
    """Shard + pack the full inputs into 8 per-core in_maps."""
    f32 = np.float32

    def packW(w, dt=NP_BF16):  # [D(=ko*ki), X] -> [ki, ko, X]
        kk, x = w.shape
        return np.ascontiguousarray(
            w.astype(f32).reshape(kk // P, P, x).transpose(1, 0, 2)).astype(dt)

    def packV(*vs):  # k vectors [D] -> [P, DT, k] fp32
        cols = [v.astype(f32).reshape(DT, P).T[:, :, None] for v in vs]
        return np.ascontiguousarray(np.concatenate(cols, axis=2))

    def packC(w):  # conv [KS, D, D] -> [DT(mo), P(ki), KS, DT(ko), P(mi)]
        w = w.astype(f32).reshape(KS, DT, P, DT, P)          # k, ko, ki, mo, mi
        return np.ascontiguousarray(w.transpose(3, 2, 0, 1, 4)).astype(NP_BF16)

    shared = {
        "keysT": packW(inputs["wm_keys"].astype(f32).T.copy()),
        "wq_ltm": packW(inputs["Wq_ltm"] * INV_SQRT),
        "wo_ltm": packW(inputs["Wo_ltm"]),
        "wq_wm": packW(inputs["Wq_wm"] * INV_SQRT),
        "wo_wm": packW(inputs["Wo_wm"]),
        "wqw_wm": packW(inputs["Wqw_wm"] * INV_SQRT),
        "wqw_ltm": packW(inputs["Wqw_ltm"] * INV_SQRT),
        "ww_cat": packW(np.concatenate([inputs["Ww_wm"], inputs["Ww_ltm"]],
                                       axis=1)),
        "gvec": packV(inputs["wg_ltm"], inputs["wg_wm"], inputs["wgw_wm"],
                      inputs["wgw_ltm"]),
        "cbias": packV(inputs["conv1_b"], inputs["conv2_b"]),
        "lnw": packV(inputs["ln_g"], inputs["ln_b"]),
        "c1w": packC(inputs["conv1_w"]),
        "c2w": packC(inputs["conv2_w"]),
    }
    mask0 = np.zeros((P, HALO), f32)
    mask1 = np.ones((P, HALO), f32)

    in_maps = []
    for b in range(B):
        cache = inputs["cache"][b].astype(f32)               # [NL, DC]
        cacheN = np.ascontiguousarray(
            cache.reshape(NLT, P, DC).transpose(1, 0, 2)).astype(NP_BF16)
        cacheT = packW(np.ascontiguousarray(cache.T))        # [P, DCT, NL]
        content = np.ascontiguousarray(inputs["wm"][b, :, :DC].astype(f32))
        contT = packW(np.ascontiguousarray(content.T))       # [P, DCT, KW]
        for h in range(2):
            if h == 0:
                xT = np.zeros((D, S_TOT), f32)
                xT[:, HALO:] = inputs["x"][b, :S_LOC].astype(f32).T
            else:
                xT = np.ascontiguousarray(
                    inputs["x"][b, S_LOC - HALO:].astype(f32).T)
            xT_p = np.ascontiguousarray(
                xT.reshape(DT, P, S_TOT).transpose(1, 0, 2))
            in_maps.append({
                "xT": xT_p, "cacheN": cacheN, "cacheT": cacheT,
                "contT": contT, "contN": content.astype(NP_BF16),
                "mask": mask0 if h == 0 else mask1, **shared,
            })
    return in_maps


def host_combine(inputs, results):
    f32 = np.float32
    out = np.empty((B, S, D), f32)
    updated_cache = np.empty((B, NL, DC), f32)
    updated_wm = np.empty((B, KW, DC + 1), f32)
    for b in range(B):
        r0, r1 = results[2 * b], results[2 * b + 1]
        out[b, :S_LOC] = r0["outT"].transpose(1, 0, 2).reshape(D, S_LOC).T
        out[b, S_LOC:] = r1["outT"].transpose(1, 0, 2).reshape(D, S_LOC).T

        Wm = r0["wcw"] + r1["wcw"]
        tot = Wm[:, DC]
        avg = Wm[:, :DC] / (tot[:, None] + EPS)
        alpha = np.clip(tot, 0.0, 1.0)[:, None]
        content = inputs["wm"][b, :, :DC].astype(f32)
        new_content = (1.0 - alpha) * content + alpha * avg
        new_valid = np.clip(inputs["wm"][b, :, DC].astype(f32) + tot, 0.0, 1.0)
        updated_wm[b] = np.concatenate([new_content, new_valid[:, None]], -1)

        Wl = r0["wcl"] + r1["wcl"]
        totl = Wl[:, DC]
        avgl = Wl[:, :DC] / (totl[:, None] + EPS)
        alphal = np.clip(totl, 0.0, 1.0)[:, None]
        updated_cache[b] = ((1.0 - alphal) * inputs["cache"][b].astype(f32)
                            + alphal * avgl)
    return out, updated_cache, updated_wm


def kernel(**inputs):
    nc = _get_nc()
    in_maps = host_prep(inputs)
    res = run_bass_kernel_spmd(nc, in_maps, core_ids=list(range(8)))
    return host_combine(inputs, res.results)


if __name__ == "__main__":
    d = np.load("/root/problem/ref_cache.npz")
    inputs = {k: d[k] for k in d.files if not k.startswith("__")}
    got = kernel(**inputs)
    exp = (d["__out"], d["__cache"], d["__wm"])
    for name, e, g in zip(("out", "cache", "wm"), exp, got):
        err = np.abs(e - g)
        rel = err.max() / (np.abs(e).max() + 1e-30)
        print(f"{name}: absmax_err={err.max():.3e} rel={rel:.3e}")


# revision 16
# speedup vs baseline: 1.0156x; 1.0156x over previous
"""Trainium2 Bass kernel for nn_DecoderCacheModel_25451976196641.

Sharding: 8 cores = 4 batches x 2 sequence halves. Each core processes
S_LOC=1024 positions plus a 12-position left halo (causal conv receptive
field: conv1 k=5 d=1 -> 4, conv2 k=5 d=2 -> 8). All activations live in
transposed layout [D(partitions), S(free)] so every x@W GEMM uses the
natural weight matrix as lhsT and the activation as the moving operand,
and conv taps become free-dim column shifts. Attention softmax runs in
[S_tile(part), N(free)] layout (logits via lhsT=q^T); the sigmoid gate and
1/denominator are folded into one per-row scalar before the attention
matrix is transposed back (PE transpose) for the read GEMM. The write
phase accumulates [gated attn]^T @ [c | 1] in PSUM, yielding both the
weighted-content numerator and the weight totals in one chain; the final
cross-half combine (tiny elementwise blend of cache/wm) happens on host.

Precision: fp32 matmuls lower to LOW_HIGH two-pass emulation (~5.5x slower
than bf16), so all large GEMMs take bf16 operands (weights cast on host,
activations shadow-cast on DVE) while PSUM accumulation, residual state,
softmax statistics, layernorm statistics and all outputs stay fp32.
"""

import sys

sys.path.insert(0, "/opt/trn_rl_repo")

import ml_dtypes
import numpy as np

import concourse.bacc as bacc
import concourse.bass as bass
import concourse.tile as tile
from concourse import mybir
from concourse.bass_utils import run_bass_kernel_spmd
from concourse.masks import make_identity

F32 = mybir.dt.float32
BF16 = mybir.dt.bfloat16
NP_BF16 = ml_dtypes.bfloat16
AF = mybir.ActivationFunctionType
ALU = mybir.AluOpType
AX = mybir.AxisListType

B, S, D, DC, NL, KW, KS = 4, 2048, 1024, 256, 768, 8, 5
P = 128
DT = D // P            # 8 partition-tiles of D
DCT = DC // P          # 2
NLT = NL // P          # 6
HALO = 12
S_LOC = S // 2         # 1024
S_TOT = S_LOC + HALO   # 1036
LN_EPS = 1e-5
EPS = 1e-6
INV_SQRT = 1.0 / np.sqrt(np.float32(DC))

# free-dim chunk lists: (start, width)
RCHUNKS = [(0, 512), (512, 512), (1024, HALO)]          # read phase, all 1036 cols
C1CHUNKS = [(4, 512), (516, 512), (1028, 8)]            # conv1 outputs
C2CHUNKS = [(12, 512), (524, 512)]                      # conv2 outputs / LN / writes
RTILES = [(i * P, P) for i in range(8)] + [(1024, HALO)]  # read-phase S-tiles
WTILES = [(HALO + i * P, P) for i in range(8)]            # write-phase S-tiles
NCHUNKS = [(0, 512), (512, 256)]                          # 768-wide logits split


def _build_nc():
    nc = bacc.Bacc("TRN2", target_bir_lowering=False, debug=False,
                   enable_asserts=False)

    def din(name, shape, dt=F32):
        return nc.dram_tensor(name, list(shape), dt, kind="ExternalInput")

    def dout(name, shape):
        return nc.dram_tensor(name, list(shape), F32, kind="ExternalOutput")

    xT = din("xT", [P, DT, S_TOT])
    cacheN = din("cacheN", [P, NLT, DC], BF16)   # cache rows on partitions
    cacheT = din("cacheT", [P, DCT, NL], BF16)   # cache^T, DC on partitions
    contT = din("contT", [P, DCT, KW], BF16)     # wm content^T
    contN = din("contN", [KW, DC], BF16)         # wm content natural
    keysT = din("keysT", [P, DCT, KW], BF16)     # wm_keys^T
    wq_ltm = din("wq_ltm", [P, DT, DC], BF16)
    wo_ltm = din("wo_ltm", [P, DCT, D], BF16)
    wq_wm = din("wq_wm", [P, DT, DC], BF16)
    wo_wm = din("wo_wm", [P, DCT, D], BF16)
    wqw_wm = din("wqw_wm", [P, DT, DC], BF16)
    wqw_ltm = din("wqw_ltm", [P, DT, DC], BF16)
    ww_cat = din("ww_cat", [P, DT, 2 * DC], BF16)   # [Ww_wm | Ww_ltm]
    gvec = din("gvec", [P, DT, 4])               # wg_ltm, wg_wm, wgw_wm, wgw_ltm
    cbias = din("cbias", [P, DT, 2])             # conv1_b, conv2_b
    lnw = din("lnw", [P, DT, 2])                 # ln_g, ln_b
    c1w = din("c1w", [DT, P, KS, DT, P], BF16)   # conv1_w packed per m-tile
    c2w = din("c2w", [DT, P, KS, DT, P], BF16)
    mask = din("mask", [P, HALO])

    outT = dout("outT", [P, DT, S_LOC])
    wcw_o = dout("wcw", [KW, DC + 1])
    wcl_o = dout("wcl", [NL, DC + 1])

    with tile.TileContext(nc) as tc:
        with (
            tc.tile_pool(name="const", bufs=1) as cst,
            tc.tile_pool(name="big", bufs=1) as big,
            tc.tile_pool(name="shadow", bufs=1) as shp,
            tc.tile_pool(name="dc", bufs=2) as dc,
            tc.tile_pool(name="attn", bufs=1) as attnp,
            tc.tile_pool(name="claug", bufs=1) as claugp,
            tc.tile_pool(name="upool", bufs=3) as up,
            tc.tile_pool(name="stp", bufs=8) as stp,
            tc.tile_pool(name="pw", bufs=4) as pw,
            tc.tile_pool(name="cwp", bufs=2) as cwp,
            tc.tile_pool(name="woutp", bufs=2) as woutp,
            tc.tile_pool(name="lnrow", bufs=4) as lnrow,
            tc.tile_pool(name="ps", bufs=1, space="PSUM") as ps,
        ):
            # ---------- constants ----------
            ident = cst.tile([P, P], BF16, tag="ident", name="ident")
            make_identity(nc, ident)
            ones_col = cst.tile([P, 1], BF16, tag="ones_col", name="ones_col")
            nc.vector.memset(ones_col, 1.0)
            ones_row = cst.tile([1, P], F32, tag="ones_row", name="ones_row")
            nc.vector.memset(ones_row, 1.0)
            eps_t = cst.tile([1, 1], F32, tag="eps_t", name="eps_t")
            nc.vector.memset(eps_t, LN_EPS)

            cacheT_sb = cst.tile([P, DCT, NL], BF16, tag="cacheT", name="cacheT_sb")
            nc.sync.dma_start(out=cacheT_sb, in_=cacheT.ap())
            cacheN_sb = cst.tile([P, NLT, DC], BF16, tag="cacheN", name="cacheN_sb")
            nc.sync.dma_start(out=cacheN_sb, in_=cacheN.ap())
            contT_sb = cst.tile([P, DCT, KW], BF16, tag="contT", name="contT_sb")
            nc.sync.dma_start(out=contT_sb, in_=contT.ap())
            contN_sb = cst.tile([KW, DC], BF16, tag="contN", name="contN_sb")
            nc.sync.dma_start(out=contN_sb, in_=contN.ap())
            keysT_sb = cst.tile([P, DCT, KW], BF16, tag="keysT", name="keysT_sb")
            nc.sync.dma_start(out=keysT_sb, in_=keysT.ap())
            gvec_sb = cst.tile([P, DT, 4], F32, tag="gvec", name="gvec_sb")
            nc.sync.dma_start(out=gvec_sb, in_=gvec.ap())
            gvec_bf = cst.tile([P, DT, 4], BF16, tag="gvec_bf", name="gvec_bf")
            nc.vector.tensor_copy(gvec_bf, gvec_sb)
            cb_sb = cst.tile([P, DT, 2], F32, tag="cb", name="cb_sb")
            nc.sync.dma_start(out=cb_sb, in_=cbias.ap())
            ln_sb = cst.tile([P, DT, 2], F32, tag="ln", name="ln_sb")
            nc.sync.dma_start(out=ln_sb, in_=lnw.ap())
            mask_sb = cst.tile([P, HALO], F32, tag="mask", name="mask_sb")
            nc.sync.dma_start(out=mask_sb, in_=mask.ap())
            u2T_sb = cst.tile([KW, S_TOT], BF16, tag="u2T", name="u2T_sb")

            # gate storage, one column per S-tile
            g_ltm = cst.tile([P, 9], F32, tag="g_ltm", name="g_ltm")
            g_wm = cst.tile([P, 9], F32, tag="g_wm", name="g_wm")
            g_wmw = cst.tile([P, 8], F32, tag="g_wmw", name="g_wmw")
            g_ltw = cst.tile([P, 8], F32, tag="g_ltw", name="g_ltw")

            # ---------- main activation buffers (fp32 state) ----------
            A = big.tile([P, DT, S_TOT], F32, tag="A", name="A")      # x -> x_ltm -> x_enh -> h
            Hb = big.tile([P, DT, S_TOT], F32, tag="Hb", name="Hb")   # h1 -> out^T
            for c0, cw in RCHUNKS:  # split so phase R1 starts on chunk 0 early
                nc.sync.dma_start(out=A[:, :, c0:c0 + cw],
                                  in_=xT.ap()[:, :, c0:c0 + cw])
            nc.vector.memset(Hb[:, :, 0:4], 0.0)  # conv1 never writes cols 0..3

            # =============================================================
            # helpers
            # =============================================================
            def cast_shadow(src, tag, cols=S_TOT):
                """bf16 shadow copy of a [P, DT, cols] fp32 buffer."""
                sh = shp.tile([P, DT, cols], BF16, tag=tag, name=f"sh_{tag}")
                for ks in range(DT):
                    nc.vector.tensor_copy(sh[:, ks, :], src[:, ks, 0:cols])
                return sh

            def proj_T(dst, w_sb, src, ksubs, chunks):
                """dst[:, m, c] = sum_ks w_sb[:, ks, m-slice]^T @ src[:, ks, c]"""
                mt = dst.shape[1]
                for c0, cw in chunks:
                    for m in range(mt):
                        pq = ps.tile([P, cw], F32, bufs=3, tag="mm", name="pq")
                        for ks in range(ksubs):
                            nc.tensor.matmul(
                                pq, lhsT=w_sb[:, ks, m * P:(m + 1) * P],
                                rhs=src[:, ks, c0:c0 + cw],
                                start=(ks == 0), stop=(ks == ksubs - 1))
                        nc.vector.tensor_copy(dst[:, m, c0:c0 + cw], pq)

            def gates(dst, src, gidx, tiles):
                """dst[:stw, i] = sigmoid(sum_d src[d, cols] * gvec[d, gidx])"""
                for i, (c0, stw) in enumerate(tiles):
                    pg = ps.tile([stw, 1], F32, bufs=3, tag="mm", name="pg")
                    for ks in range(DT):
                        nc.tensor.matmul(
                            pg, lhsT=src[:, ks, c0:c0 + stw],
                            rhs=gvec_bf[:, ks, gidx:gidx + 1],
                            start=(ks == 0), stop=(ks == DT - 1))
                    nc.scalar.activation(dst[:stw, i:i + 1], pg, AF.Sigmoid)

            def softmax_fold(psl, stw, g_col, out_u):
                """out_u = exp(psl) * (g / sum exp); |logits| < 6 so no
                max-subtraction is needed for fp32 exp."""
                den = stp.tile([stw, 1], F32, tag="st", name="den")
                nc.scalar.activation(out_u, psl, AF.Exp, accum_out=den)
                rden = stp.tile([stw, 1], F32, tag="st", name="rden")
                nc.vector.reciprocal(rden, den)
                tsc = stp.tile([stw, 1], F32, tag="st", name="tsc")
                nc.vector.tensor_mul(tsc, rden, g_col)
                nc.vector.tensor_scalar_mul(out_u, out_u, tsc)

            # =============================================================
            # Phase R1: LTM read.  A = x^T -> x_ltm^T
            # =============================================================
            Ab = cast_shadow(A, "Ab")            # bf16(x^T)
            qT = dc.tile([P, DCT, S_TOT], BF16, tag="dc", name="qT")
            wq_ltm_sb = pw.tile([P, DT, DC], BF16, tag="pw", name="wq_ltm_sb")
            nc.sync.dma_start(out=wq_ltm_sb, in_=wq_ltm.ap())
            proj_T(qT, wq_ltm_sb, Ab, DT, RCHUNKS)
            gates(g_ltm, Ab, 0, RTILES)

            uT = attnp.tile([P, NLT, S_TOT], BF16, tag="attn", name="uT")
            for sti, (c0, stw) in enumerate(RTILES):
                psl = ps.tile([stw, NL], F32, bufs=2, tag="lg", name="psl")
                for ks in range(DCT):
                    for n0, nw in NCHUNKS:
                        nc.tensor.matmul(
                            psl[:, n0:n0 + nw], lhsT=qT[:, ks, c0:c0 + stw],
                            rhs=cacheT_sb[:, ks, n0:n0 + nw],
                            start=(ks == 0), stop=(ks == DCT - 1))
                u = up.tile([stw, NL], BF16, tag="u", name="u")
                softmax_fold(psl, stw, g_ltm[:stw, sti:sti + 1], u)
                for nk in range(NLT):
                    ptr = ps.tile([P, stw], BF16, bufs=1, tag="tr", name="ptr")
                    nc.tensor.transpose(ptr, u[:, nk * P:(nk + 1) * P],
                                        ident[:stw, :stw])
                    nc.vector.tensor_copy(uT[:, nk, c0:c0 + stw], ptr)

            rgT = dc.tile([P, DCT, S_TOT], BF16, tag="dc", name="rgT")
            proj_T(rgT, cacheN_sb, uT, NLT, RCHUNKS)

            wo_ltm_sb = pw.tile([P, DCT, D], BF16, tag="pw", name="wo_ltm_sb")
            nc.sync.dma_start(out=wo_ltm_sb, in_=wo_ltm.ap())
            for c0, cw in RCHUNKS:
                for mt in range(DT):
                    pz = ps.tile([P, cw], F32, bufs=3, tag="mm", name="pz")
                    for ks in range(DCT):
                        nc.tensor.matmul(
                            pz, lhsT=wo_ltm_sb[:, ks, mt * P:(mt + 1) * P],
                            rhs=rgT[:, ks, c0:c0 + cw],
                            start=(ks == 0), stop=(ks == DCT - 1))
                    nc.vector.tensor_add(A[:, mt, c0:c0 + cw],
                                         A[:, mt, c0:c0 + cw], pz)

            # =============================================================
            # Phase R2: WM read.  A = x_ltm^T -> x_enh^T  (masked halo)
            # =============================================================
            Ab = cast_shadow(A, "Ab")            # bf16(x_ltm^T)
            q2T = dc.tile([P, DCT, S_TOT], BF16, tag="dc", name="q2T")
            wq_wm_sb = pw.tile([P, DT, DC], BF16, tag="pw", name="wq_wm_sb")
            nc.sync.dma_start(out=wq_wm_sb, in_=wq_wm.ap())
            proj_T(q2T, wq_wm_sb, Ab, DT, RCHUNKS)
            gates(g_wm, Ab, 1, RTILES)

            for sti, (c0, stw) in enumerate(RTILES):
                psl2 = ps.tile([stw, KW], F32, bufs=3, tag="mm", name="psl2")
                for ks in range(DCT):
                    nc.tensor.matmul(
                        psl2, lhsT=q2T[:, ks, c0:c0 + stw],
                        rhs=contT_sb[:, ks, :],
                        start=(ks == 0), stop=(ks == DCT - 1))
                u2 = up.tile([stw, KW], BF16, tag="u", name="u2")
                softmax_fold(psl2, stw, g_wm[:stw, sti:sti + 1], u2)
                pt2 = ps.tile([KW, stw], BF16, bufs=1, tag="tr", name="pt2")
                nc.tensor.transpose(pt2, u2, ident[:stw, :stw])
                nc.vector.tensor_copy(u2T_sb[:, c0:c0 + stw], pt2)

            r2T = dc.tile([P, DCT, S_TOT], BF16, tag="dc", name="r2T")
            for c0, cw in RCHUNKS:
                for m in range(DCT):
                    pr2 = ps.tile([P, cw], F32, bufs=3, tag="mm", name="pr2")
                    nc.tensor.matmul(pr2, lhsT=contN_sb[:, m * P:(m + 1) * P],
                                     rhs=u2T_sb[:, c0:c0 + cw],
                                     start=True, stop=True)
                    nc.vector.tensor_copy(r2T[:, m, c0:c0 + cw], pr2)

            wo_wm_sb = pw.tile([P, DCT, D], BF16, tag="pw", name="wo_wm_sb")
            nc.sync.dma_start(out=wo_wm_sb, in_=wo_wm.ap())
            for c0, cw in RCHUNKS:
                for mt in range(DT):
                    pz2 = ps.tile([P, cw], F32, bufs=3, tag="mm", name="pz2")
                    for ks in range(DCT):
                        nc.tensor.matmul(
                            pz2, lhsT=wo_wm_sb[:, ks, mt * P:(mt + 1) * P],
                            rhs=r2T[:, ks, c0:c0 + cw],
                            start=(ks == 0), stop=(ks == DCT - 1))
                    nc.vector.tensor_add(A[:, mt, c0:c0 + cw],
                                         A[:, mt, c0:c0 + cw], pz2)

            for pt in range(DT):
                nc.vector.tensor_mul(A[:, pt, 0:HALO], A[:, pt, 0:HALO],
                                     mask_sb)

            # =============================================================
            # Phase C1 / C2: causal dilated convs (bf16 GEMMs, fp32 state)
            # =============================================================
            def conv(src_sh, src, dst, w_dram, dil, chunks, cb_idx):
                w_ap = w_dram.ap()
                for mt in range(DT):
                    wt = cwp.tile([P, KS, DT, P], BF16, tag="cw", name="wt")
                    nc.sync.dma_start(out=wt, in_=w_ap[mt])
                    for c0, cw in chunks:
                        pc = ps.tile([P, cw], F32, bufs=3, tag="mm", name="pc")
                        for k in range(KS):
                            sh = (KS - 1 - k) * dil
                            for ks in range(DT):
                                nc.tensor.matmul(
                                    pc, lhsT=wt[:, k, ks, :],
                                    rhs=src_sh[:, ks, c0 - sh:c0 - sh + cw],
                                    start=(k == 0 and ks == 0),
                                    stop=(k == KS - 1 and ks == DT - 1))
                        nc.scalar.activation(dst[:, mt, c0:c0 + cw], pc,
                                             AF.Gelu_apprx_tanh,
                                             bias=cb_sb[:, mt, cb_idx:cb_idx + 1])
                        nc.vector.tensor_add(dst[:, mt, c0:c0 + cw],
                                             dst[:, mt, c0:c0 + cw],
                                             src[:, mt, c0:c0 + cw])

            Ab = cast_shadow(A, "Ab")            # bf16(x_enh^T), post-mask
            conv(Ab, A, Hb, c1w, 1, C1CHUNKS, 0)
            for pt in range(DT):
                nc.vector.tensor_mul(Hb[:, pt, 4:HALO], Hb[:, pt, 4:HALO],
                                     mask_sb[:, 4:HALO])
            Hbs = cast_shadow(Hb, "Hbs")         # bf16(h1)
            conv(Hbs, Hb, A, c2w, 2, C2CHUNKS, 1)

            # prefetch all write-phase weights during conv/LN
            wqw_wm_sb = pw.tile([P, DT, DC], BF16, tag="pw", name="wqw_wm_sb")
            nc.sync.dma_start(out=wqw_wm_sb, in_=wqw_wm.ap())
            wqw_ltm_sb = pw.tile([P, DT, DC], BF16, tag="pw", name="wqw_ltm_sb")
            nc.sync.dma_start(out=wqw_ltm_sb, in_=wqw_ltm.ap())
            ww_cat_sb = pw.tile([P, DT, 2 * DC], BF16, bufs=1, tag="pwc", name="ww_cat_sb")
            nc.sync.dma_start(out=ww_cat_sb, in_=ww_cat.ap())

            # =============================================================
            # Phase LN: Hb[:, :, 12:] = layernorm(A) = out^T
            # =============================================================
            Ab = cast_shadow(A, "Ab")            # bf16(h)
            for c0, cw in C2CHUNKS:
                pss = ps.tile([1, cw], F32, bufs=3, tag="mm", name="pss")
                for ks in range(DT):
                    nc.tensor.matmul(pss, lhsT=ones_col,
                                     rhs=Ab[:, ks, c0:c0 + cw],
                                     start=(ks == 0), stop=(ks == DT - 1))
                sq = attnp.tile([P, DT, cw], BF16, tag="attn", name="sq")
                for ks in range(DT):
                    nc.scalar.activation(sq[:, ks, :], A[:, ks, c0:c0 + cw],
                                         AF.Square)
                psq = ps.tile([1, cw], F32, bufs=3, tag="mm", name="psq")
                for ks in range(DT):
                    nc.tensor.matmul(psq, lhsT=ones_col, rhs=sq[:, ks, :],
                                     start=(ks == 0), stop=(ks == DT - 1))

                mean_r = lnrow.tile([1, cw], F32, tag="lnr", name="mean_r")
                nc.scalar.mul(mean_r, pss, 1.0 / D)
                msq_r = lnrow.tile([1, cw], F32, tag="lnr", name="msq_r")
                nc.vector.tensor_mul(msq_r, mean_r, mean_r)
                var_r = lnrow.tile([1, cw], F32, tag="lnr", name="var_r")
                nc.scalar.mul(var_r, psq, 1.0 / D)
                nc.vector.tensor_sub(var_r, var_r, msq_r)
                rstd_r = lnrow.tile([1, cw], F32, tag="lnr", name="rstd_r")
                nc.scalar.activation(rstd_r, var_r, AF.Sqrt, bias=eps_t)
                nc.vector.reciprocal(rstd_r, rstd_r)

                pmb = ps.tile([P, cw], F32, bufs=2, tag="lg", name="pmb")
                nc.tensor.matmul(pmb, lhsT=ones_row, rhs=mean_r,
                                 start=True, stop=True)
                prb = ps.tile([P, cw], F32, bufs=2, tag="lg", name="prb")
                nc.tensor.matmul(prb, lhsT=ones_row, rhs=rstd_r,
                                 start=True, stop=True)
                for mt in range(DT):
                    nc.vector.tensor_sub(Hb[:, mt, c0:c0 + cw],
                                         A[:, mt, c0:c0 + cw], pmb)
                    nc.vector.tensor_mul(Hb[:, mt, c0:c0 + cw],
                                         Hb[:, mt, c0:c0 + cw], prb)
                    nc.vector.tensor_scalar(
                        out=Hb[:, mt, c0:c0 + cw],
                        in0=Hb[:, mt, c0:c0 + cw],
                        scalar1=ln_sb[:, mt, 0:1], scalar2=ln_sb[:, mt, 1:2],
                        op0=ALU.mult, op1=ALU.add)
                nc.gpsimd.dma_start(
                    out=outT.ap()[:, :, c0 - HALO:c0 - HALO + cw],
                    in_=Hb[:, :, c0:c0 + cw])

            # =============================================================
            # Phase W1+W2: WM + LTM write partials (merged)
            # =============================================================
            Hbs = cast_shadow(Hb, "Hbs")         # bf16(out^T); lhsT and rhs
            qwT = dc.tile([P, DCT, S_TOT], BF16, tag="dc", name="qwT")
            proj_T(qwT, wqw_wm_sb, Hbs, DT, C2CHUNKS)
            qlT = dc.tile([P, DCT, S_TOT], BF16, tag="dc", name="qlT")
            proj_T(qlT, wqw_ltm_sb, Hbs, DT, C2CHUNKS)

            # fused write gates: one chain, two output columns
            g_w2 = cst.tile([P, 8, 2], F32, tag="g_w2", name="g_w2")
            for sti, (c0, stw) in enumerate(WTILES):
                pgw = ps.tile([stw, 2], F32, bufs=3, tag="mm", name="pgw")
                for ks in range(DT):
                    nc.tensor.matmul(
                        pgw, lhsT=Hbs[:, ks, c0:c0 + stw],
                        rhs=gvec_bf[:, ks, 2:4],
                        start=(ks == 0), stop=(ks == DT - 1))
                nc.scalar.activation(g_w2[:stw, sti, :], pgw, AF.Sigmoid)

            uw_all = cst.tile([P, 8, KW], BF16, tag="uw_all", name="uw_all")
            ul_all = attnp.tile([P, 8, NL], BF16, tag="attn", name="ul_all")
            caug = claugp.tile([P, 8, DC + 1], BF16, tag="claug", name="caug")
            claug = claugp.tile([P, 8, DC + 1], BF16, tag="claug2", name="claug2")
            for sti, (c0, stw) in enumerate(WTILES):
                pcc = ps.tile([P, 2 * DC], F32, bufs=2, tag="lg", name="pcc")
                for ks in range(DT):
                    nc.tensor.matmul(pcc, lhsT=Hbs[:, ks, c0:c0 + stw],
                                     rhs=ww_cat_sb[:, ks, :],
                                     start=(ks == 0), stop=(ks == DT - 1))
                nc.vector.tensor_copy(caug[:, sti, 0:DC], pcc[:, 0:DC])
                nc.vector.memset(caug[:, sti, DC:DC + 1], 1.0)
                nc.vector.tensor_copy(claug[:, sti, 0:DC], pcc[:, DC:2 * DC])
                nc.vector.memset(claug[:, sti, DC:DC + 1], 1.0)

                plw = ps.tile([stw, KW], F32, bufs=3, tag="mm", name="plw")
                for ks in range(DCT):
                    nc.tensor.matmul(plw, lhsT=qwT[:, ks, c0:c0 + stw],
                                     rhs=keysT_sb[:, ks, :],
                                     start=(ks == 0), stop=(ks == DCT - 1))
                softmax_fold(plw, stw, g_w2[:stw, sti, 0:1],
                             uw_all[:, sti, :])

                pll = ps.tile([stw, NL], F32, bufs=2, tag="lg", name="pll")
                for ks in range(DCT):
                    for n0, nw in NCHUNKS:
                        nc.tensor.matmul(
                            pll[:, n0:n0 + nw], lhsT=qlT[:, ks, c0:c0 + stw],
                            rhs=cacheT_sb[:, ks, n0:n0 + nw],
                            start=(ks == 0), stop=(ks == DCT - 1))
                softmax_fold(pll, stw, g_w2[:stw, sti, 1:2],
                             ul_all[:, sti, :])

            pacc = ps.tile([KW, DC + 1], F32, bufs=3, tag="mm", name="pacc")
            for sti in range(8):
                nc.tensor.matmul(pacc, lhsT=uw_all[:, sti, :],
                                 rhs=caug[:, sti, :],
                                 start=(sti == 0), stop=(sti == 7))
            wcw_sb = woutp.tile([KW, DC + 1], F32, tag="wout", name="wcw_sb")
            nc.vector.tensor_copy(wcw_sb, pacc)
            nc.sync.dma_start(out=wcw_o.ap(), in_=wcw_sb)

            for mt6 in range(NLT):
                pacl = ps.tile([P, DC + 1], F32, bufs=3, tag="mm", name="pacl")
                for sti in range(8):
                    nc.tensor.matmul(pacl,
                                     lhsT=ul_all[:, sti, mt6 * P:(mt6 + 1) * P],
                                     rhs=claug[:, sti, :],
                                     start=(sti == 0), stop=(sti == 7))
                wcl_sb = woutp.tile([P, DC + 1], F32, tag="wout",
                                    name="wcl_sb")
                nc.vector.tensor_copy(wcl_sb, pacl)
                nc.sync.dma_start(out=wcl_o.ap()[mt6 * P:(mt6 + 1) * P, :],
                                  in_=wcl_sb)

    nc.compile()
    return nc


_NC = None


def _get_nc():
    global _NC
    if _NC is None:
        _NC = _build_nc()
    return _NC


def host_prep(inputs):
    """Shard + pack the full inputs into 8 per-core in_maps."""
    f32 = np.float32

    def packW(w, dt=NP_BF16):  # [D(=ko*ki), X] -> [ki, ko, X]
        kk, x = w.shape
        return np.ascontiguousarray(
            w.astype(f32).reshape(kk // P, P, x).transpose(1, 0, 2)).astype(dt)

    def packV(*vs):  # k vectors [D] -> [P, DT, k] fp32
        cols = [v.astype(f32).reshape(DT, P).T[:, :, None] for v in vs]
        return np.ascontiguousarray(np.concatenate(cols, axis=2))

    def packC(w):  # conv [KS, D, D] -> [DT(mo), P(ki), KS, DT(ko), P(mi)]
        w = w.astype(f32).reshape(KS, DT, P, DT, P)          # k, ko, ki, mo, mi
        return np.ascontiguousarray(w.transpose(3, 2, 0, 1, 4)).astype(NP_BF16)

    shared = {
        "keysT": packW(inputs["wm_keys"].astype(f32).T.copy()),
        "wq_ltm": packW(inputs["Wq_ltm"] * INV_SQRT),
        "wo_ltm": packW(inputs["Wo_ltm"]),
        "wq_wm": packW(inputs["Wq_wm"] * INV_SQRT),
        "wo_wm": packW(inputs["Wo_wm"]),
        "wqw_wm": packW(inputs["Wqw_wm"] * INV_SQRT),
        "wqw_ltm": packW(inputs["Wqw_ltm"] * INV_SQRT),
        "ww_cat": packW(np.concatenate([inputs["Ww_wm"], inputs["Ww_ltm"]],
                                       axis=1)),
        "gvec": packV(inputs["wg_ltm"], inputs["wg_wm"], inputs["wgw_wm"],
                      inputs["wgw_ltm"]),
        "cbias": packV(inputs["conv1_b"], inputs["conv2_b"]),
        "lnw": packV(inputs["ln_g"], inputs["ln_b"]),
        "c1w": packC(inputs["conv1_w"]),
        "c2w": packC(inputs["conv2_w"]),
    }
    mask0 = np.zeros((P, HALO), f32)
    mask1 = np.ones((P, HALO), f32)

    in_maps = []
    for b in range(B):
        cache = inputs["cache"][b].astype(f32)               # [NL, DC]
        cacheN = np.ascontiguousarray(
            cache.reshape(NLT, P, DC).transpose(1, 0, 2)).astype(NP_BF16)
        cacheT = packW(np.ascontiguousarray(cache.T))        # [P, DCT, NL]
        content = np.ascontiguousarray(inputs["wm"][b, :, :DC].astype(f32))
        contT = packW(np.ascontiguousarray(content.T))       # [P, DCT, KW]
        for h in range(2):
            if h == 0:
                xT = np.zeros((D, S_TOT), f32)
                xT[:, HALO:] = inputs["x"][b, :S_LOC].astype(f32).T
            else:
                xT = np.ascontiguousarray(
                    inputs["x"][b, S_LOC - HALO:].astype(f32).T)
            xT_p = np.ascontiguousarray(
                xT.reshape(DT, P, S_TOT).transpose(1, 0, 2))
            in_maps.append({
                "xT": xT_p, "cacheN": cacheN, "cacheT": cacheT,
                "contT": contT, "contN": content.astype(NP_BF16),
                "mask": mask0 if h == 0 else mask1, **shared,
            })
    return in_maps


def host_combine(inputs, results):
    f32 = np.float32
    out = np.empty((B, S, D), f32)
    updated_cache = np.empty((B, NL, DC), f32)
    updated_wm = np.empty((B, KW, DC + 1), f32)
    for b in range(B):
        r0, r1 = results[2 * b], results[2 * b + 1]
        out[b, :S_LOC] = r0["outT"].transpose(1, 0, 2).reshape(D, S_LOC).T
        out[b, S_LOC:] = r1["outT"].transpose(1, 0, 2).reshape(D, S_LOC).T

        Wm = r0["wcw"] + r1["wcw"]
        tot = Wm[:, DC]
        avg = Wm[:, :DC] / (tot[:, None] + EPS)
        alpha = np.clip(tot, 0.0, 1.0)[:, None]
        content = inputs["wm"][b, :, :DC].astype(f32)
        new_content = (1.0 - alpha) * content + alpha * avg
        new_valid = np.clip(inputs["wm"][b, :, DC].astype(f32) + tot, 0.0, 1.0)
        updated_wm[b] = np.concatenate([new_content, new_valid[:, None]], -1)

        Wl = r0["wcl"] + r1["wcl"]
        totl = Wl[:, DC]
        avgl = Wl[:, :DC] / (totl[:, None] + EPS)
        alphal = np.clip(totl, 0.0, 1.0)[:, None]
        updated_cache[b] = ((1.0 - alphal) * inputs["cache"][b].astype(f32)
                            + alphal * avgl)
    return out, updated_cache, updated_wm


def kernel(**inputs):
    nc = _get_nc()
    in_maps = host_prep(inputs)
    res = run_bass_kernel_spmd(nc, in_maps, core_ids=list(range(8)))
    return host_combine(inputs, res.results)


if __name__ == "__main__":
    d = np.load("/root/problem/ref_cache.npz")
    inputs = {k: d[k] for k in d.files if not k.startswith("__")}
    got = kernel(**inputs)
    exp = (d["__out"], d["__cache"], d["__wm"])
    for name, e, g in zip(("out", "cache", "wm"), exp, got):
        err = np.abs(e - g)
        rel = err.max() / (np.abs(e).max() + 1e-30)
        print(f"{name}: absmax_err={err.max():.3e} rel={rel:.3e}")
